# revision 26
# baseline (speedup 1.0000x reference)
"""CrossEntropyLossWithGaussianSmoothedLabels on 8 TRN2 NeuronCores.

Math: the reference's scatter-built smoothed label at class j is exactly
w[|j-t|] for |j-t|<=3 (w = [1, e^-.5, e^-1, e^-2]); clamped writes are always
overwritten by the nearer-distance write. So

  loss = mean_r( W_r * logsumexp(x_r) - sum_o w[|o|] * x_r[t_r+o] )

with W_r = sum of valid window weights. The gather term runs on the
TensorEngine without any per-row gather:

  sum_r sum_o w[o] x[r, t_r+o] = sum_{|m-n|<=3} (H^T X)[m, n] * w[n-m]

where H is the one-hot target matrix; H^T X accumulates in PSUM via 6 banded
128x128 bf16 matmuls per 128-row tile. logsumexp is max-free (|x| < 6).

Key layout tricks (v4):
 * The whole per-core prediction slab (11.8 MB fp32) stays SBUF-resident, so
   the HBM stream free-runs at line rate with no consumer backpressure.
 * The PE's moving operand is a stride-2 bfloat16 ALIAS of the fp32 slab
   (the high 2 bytes of an f32 are its bf16 truncation) - the 13us DVE cast
   disappears entirely. One-hot H is built in bf16 to match.
 * exp runs on ACT in two flavors balanced across engines: per-tile exp with
   the fused accumulator (A-tiles, scalar-side reduce) and batched 4/2-tile
   exp -> f16 with a segmented DVE tensor_reduce (B-chunks). The stream tail
   alternates A/B2 so the scalar keeps pace with the DMA cadence.
 * The ACT table set is forced to natural_log_exp_and_others (exp+ln+copy in
   one set), so the kernel pays exactly one 1.28us table load, never a
   reload before the final Ln.
 * The first two x-chunks are issued from the ACT engine's HWDGE queue whose
   program entry runs ~1.3us before the Sync engine's, buying the whole
   stream that head start.
"""

import math
from contextlib import ExitStack

import numpy as np

import concourse.bacc as bacc
import concourse.hw_specs as hw_specs
from concourse import mybir
from concourse.bass_utils import run_bass_kernel_spmd

P = 128
C = 722
NCORES = 8
ROWS = 16 * 2048
RPC = ROWS // NCORES   # 4096 rows per core
NT = RPC // P          # 32 row-tiles per core
NB = 6
BLK = [0, 124, 248, 372, 496, 594]
URANGES = [(0, 124), (124, 248), (248, 372), (372, 496), (496, 594), (594, 722)]
WDEC = [1.0, math.exp(-0.5), math.exp(-1.0), math.exp(-2.0)]

# x-DMA chunks (tile_start, n_tiles)
CHUNKS = [(0, 1), (1, 1), (2, 2), (4, 4), (8, 4), (12, 4), (16, 4), (20, 2),
          (22, 2), (24, 2), (26, 2), (28, 2), (30, 1), (31, 1)]
# batched-exp chunks (rest are per-tile A chunks with the ACT accumulator)
B_CHUNKS = [3, 4, 5, 6, 7, 10]       # tiles 4..19 (4-wide) + 20-21, 26-27
B_ORD = {c: i + 1 for i, c in enumerate(B_CHUNKS)}
# DVE reduces B-chunk b while processing chunk RED_AT^-1(b)
RED_AT = {5: 1, 6: 2, 7: 3, 8: 4, 9: 5, 11: 6}
ESLOTS = 2

f32 = mybir.dt.float32
f16 = mybir.dt.float16
bf16 = mybir.dt.bfloat16
i32 = mybir.dt.int32

_orig_tables = hw_specs.get_activation_tables


def _forced_tables(arch):
    """Only natural_log_exp_and_others is pickable: exp, ln and copy live in
    one set, so the kernel loads ACT tables exactly once."""
    tabs = _orig_tables(arch)
    return {
        name: (funcs if name == "natural_log_exp_and_others" else set())
        for name, funcs in tabs.items()
    }


def _band_masks() -> np.ndarray:
    """[128, 6*128] band weights, each global band entry owned by exactly one
    block (by min(m,n) ownership range)."""
    m = np.zeros((P, NB * P), np.float32)
    for b in range(NB):
        s = BLK[b]
        lo, hi = URANGES[b]
        for i in range(P):
            for o in range(-3, 4):
                j = i + o
                if 0 <= j < P:
                    mg, ng = s + i, s + j
                    if mg < C and ng < C and lo <= min(mg, ng) < hi:
                        m[i, b * P + j] = WDEC[abs(o)]
    return m


def _chunk_of(tile: int) -> int:
    for c, (s, l) in enumerate(CHUNKS):
        if s <= tile < s + l:
            return c
    raise ValueError(tile)


def _build(rpc: int):
    nt = rpc // P
    assert nt == NT
    bacc.get_activation_tables = _forced_tables
    try:
        return _build_inner(rpc, nt)
    finally:
        bacc.get_activation_tables = _orig_tables


def _build_inner(rpc: int, nt: int):
    nc = bacc.Bacc(
        "TRN2", target_bir_lowering=False, debug=False, num_devices=NCORES
    )
    AF = mybir.ActivationFunctionType
    OP = mybir.AluOpType

    pred = nc.dram_tensor("prediction", [rpc, C], f32, kind="ExternalInput").ap()
    tgt = nc.dram_tensor("target_pair", [P, nt, 2], i32, kind="ExternalInput").ap()
    band = nc.dram_tensor("band", [P, NB, P], f32, kind="ExternalInput").ap()
    out = nc.dram_tensor("out", [P, nt + 1], f32, kind="ExternalOutput").ap()

    x_all_h = nc.alloc_sbuf_tensor("x_all", [P, nt, C], f32)
    x_all = x_all_h.ap()
    # bf16 alias of the same bytes: element 2j+1 is the high half of f32 j,
    # i.e. x truncated to bf16 - the PE streams it directly, no cast pass
    x_bf = nc.alloc_sbuf_tensor_at(
        "x_bf", [P, nt, C, 2], bf16, offset=nc.lookup_mloc(x_all_h).addr
    ).ap()

    h_all = nc.alloc_sbuf_tensor("h_all", [P, nt, C], bf16).ap()
    esc_a = nc.alloc_sbuf_tensor("esc_a", [P, 2, C], f16).ap()
    esc_b = nc.alloc_sbuf_tensor("esc_b", [P, ESLOTS, 4, C], f16).ap()
    iota_i = nc.alloc_sbuf_tensor("iota_i", [P, C], i32).ap()
    iota_h = nc.alloc_sbuf_tensor("iota_h", [P, C], f16).ap()
    t_pair = nc.alloc_sbuf_tensor("t_pair", [P, nt, 2], i32).ap()
    t_f = nc.alloc_sbuf_tensor("t_f", [P, nt], f32).ap()
    band_sb = nc.alloc_sbuf_tensor("band_sb", [P, NB, P], f32).ap()
    sumexp_sb = nc.alloc_sbuf_tensor("sumexp_sb", [P, nt], f32).ap()
    lse = nc.alloc_sbuf_tensor("lse", [P, nt], f32).ap()
    wr = nc.alloc_sbuf_tensor("wr", [P, nt], f32).ap()
    wtmp = nc.alloc_sbuf_tensor("wtmp", [P, nt], f32).ap()
    junk = nc.alloc_sbuf_tensor("junk", [P, NB, P], f32).ap()
    warm_t = nc.alloc_sbuf_tensor("warm_t", [P, 4], f32).ap()
    outsb = nc.alloc_sbuf_tensor("outsb", [P, nt + 1], f32).ap()

    psum = nc.alloc_psum_tensor("psumblk", [P, NB, 512], f32).ap()

    with (
        nc.Block() as block,
        ExitStack() as _sems,
        nc.semaphore("t_sem") as t_sem,
        nc.semaphore("warm_sem") as warm_sem,
        nc.semaphore("band_sem") as band_sem,
        nc.semaphore("g_iota") as g_iota,
        nc.semaphore("h_dve") as h_dve,
        nc.semaphore("escb") as escb,
        nc.semaphore("act_sem") as act_sem,
        nc.semaphore("bred") as bred,
        nc.semaphore("pe_tile") as pe_tile,
        nc.semaphore("lnsem") as lnsem,
        nc.semaphore("dvec") as dvec,
        nc.semaphore("vfin") as vfin,
        nc.semaphore("odma") as odma,
    ):
        cks = [_sems.enter_context(nc.semaphore(f"ck{c}")) for c in range(len(CHUNKS))]

        def load_chunk(eng, c):
            s, l = CHUNKS[c]
            src = pred[s * P:(s + l) * P, :].rearrange("(k p) c -> p k c", p=P)
            eng.dma_start(out=x_all[:, s:s + l, :], in_=src).then_inc(cks[c], 16)

        @block.sync
        def _(sync):
            load_chunk(sync, 0)
            load_chunk(sync, 1)
            sync.dma_start(out=t_pair, in_=tgt).then_inc(t_sem, 16)
            for c in range(2, len(CHUNKS)):
                load_chunk(sync, c)
            sync.wait_ge(vfin, 1)
            sync.dma_start(out=out, in_=outsb).then_inc(odma, 16)
            sync.wait_ge(odma, 16)

        @block.gpsimd
        def _(gpsimd):
            gpsimd.memset(warm_t, 0.0).then_inc(warm_sem, 1)
            gpsimd.iota(iota_i, pattern=[[1, C]], base=0,
                        channel_multiplier=0).then_inc(g_iota, 1)
            gpsimd.dma_start(out=band_sb, in_=band).then_inc(band_sem, 16)

        @block.scalar
        def _(scalar):
            scalar.wait_ge(warm_sem, 1)
            scalar.activation(out=warm_t[:, 0:1], in_=warm_t[:, 2:3], func=AF.Exp)
            a_idx = 0
            for c, (s, l) in enumerate(CHUNKS):
                scalar.wait_ge(cks[c], 16)
                if c in B_ORD:
                    b = B_ORD[c]
                    slot = (b - 1) % ESLOTS
                    if b > ESLOTS:
                        scalar.wait_ge(bred, b - ESLOTS)
                    scalar.activation(
                        out=esc_b[:, slot, 0:l, :], in_=x_all[:, s:s + l, :],
                        func=AF.Exp,
                    ).then_inc(escb, 1)
                else:
                    for i in range(s, s + l):
                        if a_idx >= 2:
                            # esc_a is double-buffered: <=2 exps in flight
                            scalar.wait_ge(act_sem, a_idx - 1)
                        scalar.activation(
                            out=esc_a[:, a_idx % 2, :], in_=x_all[:, i, :],
                            func=AF.Exp,
                            accum_out=sumexp_sb[:, i:i + 1],
                        ).then_inc(act_sem, 1)
                        a_idx += 1
            scalar.wait_ge(act_sem, a_idx)          # all accumulates landed
            scalar.wait_ge(bred, len(B_CHUNKS))     # all B sums landed
            scalar.activation(out=lse, in_=sumexp_sb, func=AF.Ln).then_inc(
                lnsem, 1)

        @block.vector
        def _(vector):
            ndve = 0

            def dv(ins):
                nonlocal ndve
                ndve += 1
                return ins.then_inc(dvec, 1)

            def dwait():
                vector.wait_ge(dvec, ndve)

            vector.wait_ge(t_sem, 16)
            dv(vector.tensor_copy(out=t_f, in_=t_pair[:, :, 0]))
            vector.wait_ge(g_iota, 1)
            dv(vector.tensor_copy(out=iota_h, in_=iota_i))
            dv(vector.memset(wr, 1.0))
            for d in (1, 2, 3):
                dwait()
                dv(vector.tensor_scalar(
                    out=wtmp, in0=t_f, scalar1=d - 0.5, scalar2=WDEC[d],
                    op0=OP.is_ge, op1=OP.mult))
                dwait()
                dv(vector.tensor_tensor(out=wr, in0=wr, in1=wtmp, op=OP.add))
                dwait()
                dv(vector.tensor_scalar(
                    out=wtmp, in0=t_f, scalar1=(C - 1 - d) + 0.5,
                    scalar2=WDEC[d], op0=OP.is_le, op1=OP.mult))
                dwait()
                dv(vector.tensor_tensor(out=wr, in0=wr, in1=wtmp, op=OP.add))
            dwait()  # preamble fully retired before the loop reads iota_h/t_f

            for c, (s, l) in enumerate(CHUNKS):
                vector.wait_ge(cks[c], 16)
                for i in range(s, s + l):
                    vector.tensor_scalar(
                        out=h_all[:, i, :], in0=iota_h,
                        scalar1=t_f[:, i:i + 1], scalar2=None,
                        op0=OP.is_equal, op1=OP.bypass,
                    ).then_inc(h_dve, 1)
                if c in RED_AT:
                    b = RED_AT[c]
                    bc = B_CHUNKS[b - 1]
                    bs, bl = CHUNKS[bc]
                    vector.wait_ge(escb, b)
                    vector.tensor_reduce(
                        out=sumexp_sb[:, bs:bs + bl],
                        in_=esc_b[:, (b - 1) % ESLOTS, 0:bl, :],
                        axis=mybir.AxisListType.X, op=OP.add,
                    ).then_inc(bred, 1)

            vector.wait_ge(pe_tile, nt)
            vector.wait_ge(band_sem, 16)
            dv(vector.tensor_tensor(
                out=junk, in0=psum[:, :, 0:P], in1=band_sb, op=OP.mult))
            dwait()
            dv(vector.tensor_reduce(
                out=outsb[:, nt:nt + 1],
                in_=junk.rearrange("p b n -> p (b n)"),
                axis=mybir.AxisListType.X, op=OP.add))
            dwait()  # G landed before the out DMA reads outsb
            vector.wait_ge(lnsem, 1)
            vector.tensor_tensor(
                out=outsb[:, 0:nt], in0=wr, in1=lse, op=OP.mult
            ).then_inc(vfin, 1)

        @block.tensor
        def _(pe):
            for i in range(nt):
                c = _chunk_of(i)
                s, l = CHUNKS[c]
                if i == s:
                    pe.wait_ge(cks[c], 16)
                    pe.wait_ge(h_dve, s + l)
                for b in range(NB):
                    bs = BLK[b]
                    mm = pe.matmul(
                        psum[:, b, 0:P], h_all[:, i, bs:bs + P],
                        x_bf[:, i, bs:bs + P, 1],
                        start=(i == 0), stop=(i == nt - 1),
                    )
                mm.then_inc(pe_tile, 1)

    nc.compile()
    return nc


def _shard_inputs(prediction: np.ndarray, target: np.ndarray, rpc: int, ncores: int):
    pred = np.ascontiguousarray(np.asarray(prediction, dtype=np.float32)).reshape(-1, C)
    tgt = np.ascontiguousarray(np.asarray(target)).reshape(-1)
    if tgt.dtype == np.int64:
        tgt_pair = tgt.view(np.int32).reshape(-1, 2)  # little-endian low word first
    else:
        lo = tgt.astype(np.int32, copy=False)
        tgt_pair = np.stack([lo, np.zeros_like(lo)], axis=1)
    nt = rpc // P
    band = _band_masks().reshape(P, NB, P)
    in_maps = []
    for c in range(ncores):
        sl = slice(c * rpc, (c + 1) * rpc)
        in_maps.append({
            "prediction": pred[sl],
            "target_pair": np.ascontiguousarray(
                tgt_pair[sl].reshape(nt, P, 2).transpose(1, 0, 2)),
            "band": band,
        })
    return in_maps


def _host_combine(results, nt: int) -> np.float32:
    tot = 0.0
    nrows = 0
    for r in results:
        o = np.asarray(r["out"], dtype=np.float64)
        tot += o[:, :nt].sum() - o[:, nt:].sum()
        nrows += P * nt
    return np.float32(tot / nrows)


def kernel(prediction: np.ndarray, target: np.ndarray, _trace: bool = False):
    nc = _build(RPC)
    in_maps = _shard_inputs(prediction, target, RPC, NCORES)
    res = run_bass_kernel_spmd(
        nc, in_maps, core_ids=list(range(NCORES)), trace=_trace
    )
    loss = _host_combine(res.results, RPC // P)
    if _trace:
        return loss, res
    return loss


# revision 28
# speedup vs baseline: 1.0537x; 1.0537x over previous
"""CrossEntropyLossWithGaussianSmoothedLabels on 8 TRN2 NeuronCores.

Math: the reference's scatter-built smoothed label at class j is exactly
w[|j-t|] for |j-t|<=3 (w = [1, e^-.5, e^-1, e^-2]); clamped writes are always
overwritten by the nearer-distance write. So

  loss = mean_r( W_r * logsumexp(x_r) - sum_o w[|o|] * x_r[t_r+o] )

with W_r = sum of valid window weights. The gather term runs on the
TensorEngine without any per-row gather:

  sum_r sum_o w[o] x[r, t_r+o] = sum_{|m-n|<=3} (H^T X)[m, n] * w[n-m]

where H is the one-hot target matrix; H^T X accumulates in PSUM via 6 banded
128x128 bf16 matmuls per 128-row tile. logsumexp is max-free (|x| < 6).

Key layout tricks (v4):
 * The whole per-core prediction slab (11.8 MB fp32) stays SBUF-resident, so
   the HBM stream free-runs at line rate with no consumer backpressure.
 * The PE's moving operand is a stride-2 bfloat16 ALIAS of the fp32 slab
   (the high 2 bytes of an f32 are its bf16 truncation) - the 13us DVE cast
   disappears entirely. One-hot H is built in bf16 to match.
 * exp runs on ACT in two flavors balanced across engines: per-tile exp with
   the fused accumulator (A-tiles, scalar-side reduce) and batched 4/2-tile
   exp -> f16 with a segmented DVE tensor_reduce (B-chunks). The stream tail
   alternates A/B2 so the scalar keeps pace with the DMA cadence.
 * The ACT table set is forced to natural_log_exp_and_others (exp+ln+copy in
   one set), so the kernel pays exactly one 1.28us table load, never a
   reload before the final Ln.
 * The first two x-chunks are issued from the ACT engine's HWDGE queue whose
   program entry runs ~1.3us before the Sync engine's, buying the whole
   stream that head start.
"""

import math
from contextlib import ExitStack

import numpy as np

import concourse.bacc as bacc
import concourse.hw_specs as hw_specs
from concourse import mybir
from concourse.bass_utils import run_bass_kernel_spmd

P = 128
C = 722
NCORES = 8
ROWS = 16 * 2048
RPC = ROWS // NCORES   # 4096 rows per core
NT = RPC // P          # 32 row-tiles per core
NB = 6
BLK = [0, 124, 248, 372, 496, 594]
URANGES = [(0, 124), (124, 248), (248, 372), (372, 496), (496, 594), (594, 722)]
WDEC = [1.0, math.exp(-0.5), math.exp(-1.0), math.exp(-2.0)]

# x-DMA chunks (tile_start, n_tiles)
CHUNKS = [(0, 1), (1, 1), (2, 2), (4, 4), (8, 4), (12, 4), (16, 4), (20, 2),
          (22, 2), (24, 2), (26, 2), (28, 2), (30, 1), (31, 1)]
# batched-exp chunks (rest are per-tile A chunks with the ACT accumulator)
B_CHUNKS = [3, 4, 5, 6, 7, 10]       # tiles 4..19 (4-wide) + 20-21, 26-27
B_ORD = {c: i + 1 for i, c in enumerate(B_CHUNKS)}
# DVE reduces B-chunk b while processing chunk RED_AT^-1(b)
RED_AT = {5: 1, 6: 2, 7: 3, 8: 4, 9: 5, 11: 6}
ESLOTS = 2

f32 = mybir.dt.float32
f16 = mybir.dt.float16
bf16 = mybir.dt.bfloat16
i32 = mybir.dt.int32

_orig_tables = hw_specs.get_activation_tables


def _forced_tables(arch):
    """Only natural_log_exp_and_others is pickable: exp, ln and copy live in
    one set, so the kernel loads ACT tables exactly once."""
    tabs = _orig_tables(arch)
    return {
        name: (funcs if name == "natural_log_exp_and_others" else set())
        for name, funcs in tabs.items()
    }


def _band_masks() -> np.ndarray:
    """[128, 6*128] band weights, each global band entry owned by exactly one
    block (by min(m,n) ownership range)."""
    m = np.zeros((P, NB * P), np.float32)
    for b in range(NB):
        s = BLK[b]
        lo, hi = URANGES[b]
        for i in range(P):
            for o in range(-3, 4):
                j = i + o
                if 0 <= j < P:
                    mg, ng = s + i, s + j
                    if mg < C and ng < C and lo <= min(mg, ng) < hi:
                        m[i, b * P + j] = WDEC[abs(o)]
    return m


def _chunk_of(tile: int) -> int:
    for c, (s, l) in enumerate(CHUNKS):
        if s <= tile < s + l:
            return c
    raise ValueError(tile)


def _build(rpc: int):
    nt = rpc // P
    assert nt == NT
    bacc.get_activation_tables = _forced_tables
    try:
        return _build_inner(rpc, nt)
    finally:
        bacc.get_activation_tables = _orig_tables


def _build_inner(rpc: int, nt: int):
    nc = bacc.Bacc(
        "TRN2", target_bir_lowering=False, debug=False, num_devices=NCORES
    )
    AF = mybir.ActivationFunctionType
    OP = mybir.AluOpType

    pred = nc.dram_tensor("prediction", [rpc, C], f32, kind="ExternalInput").ap()
    tgt = nc.dram_tensor("target_pair", [P, nt, 2], i32, kind="ExternalInput").ap()
    band = nc.dram_tensor("band", [P, NB, P], f32, kind="ExternalInput").ap()
    out = nc.dram_tensor("out", [P, nt + 1], f32, kind="ExternalOutput").ap()

    x_all_h = nc.alloc_sbuf_tensor("x_all", [P, nt, C], f32)
    x_all = x_all_h.ap()
    # bf16 alias of the same bytes: element 2j+1 is the high half of f32 j,
    # i.e. x truncated to bf16 - the PE streams it directly, no cast pass
    x_bf = nc.alloc_sbuf_tensor_at(
        "x_bf", [P, nt, C, 2], bf16, offset=nc.lookup_mloc(x_all_h).addr
    ).ap()

    h_all = nc.alloc_sbuf_tensor("h_all", [P, nt, C], bf16).ap()
    esc_a = nc.alloc_sbuf_tensor("esc_a", [P, 2, C], f16).ap()
    esc_b = nc.alloc_sbuf_tensor("esc_b", [P, ESLOTS, 4, C], f16).ap()
    iota_i = nc.alloc_sbuf_tensor("iota_i", [P, C], i32).ap()
    iota_h = nc.alloc_sbuf_tensor("iota_h", [P, C], f16).ap()
    t_pair = nc.alloc_sbuf_tensor("t_pair", [P, nt, 2], i32).ap()
    t_f = nc.alloc_sbuf_tensor("t_f", [P, nt], f32).ap()
    band_sb = nc.alloc_sbuf_tensor("band_sb", [P, NB, P], f32).ap()
    sumexp_sb = nc.alloc_sbuf_tensor("sumexp_sb", [P, nt], f32).ap()
    lse = nc.alloc_sbuf_tensor("lse", [P, nt], f32).ap()
    wr = nc.alloc_sbuf_tensor("wr", [P, nt], f32).ap()
    wtmp = nc.alloc_sbuf_tensor("wtmp", [P, nt], f32).ap()
    junk = nc.alloc_sbuf_tensor("junk", [P, NB, P], f32).ap()
    warm_t = nc.alloc_sbuf_tensor("warm_t", [P, 4], f32).ap()
    outsb = nc.alloc_sbuf_tensor("outsb", [P, nt + 1], f32).ap()

    psum = nc.alloc_psum_tensor("psumblk", [P, NB, 512], f32).ap()

    with (
        nc.Block() as block,
        ExitStack() as _sems,
        nc.semaphore("t_sem") as t_sem,
        nc.semaphore("warm_sem") as warm_sem,
        nc.semaphore("band_sem") as band_sem,
        nc.semaphore("g_iota") as g_iota,
        nc.semaphore("h_dve") as h_dve,
        nc.semaphore("escb") as escb,
        nc.semaphore("act_sem") as act_sem,
        nc.semaphore("bred") as bred,
        nc.semaphore("pe_tile") as pe_tile,
        nc.semaphore("lnsem") as lnsem,
        nc.semaphore("dvec") as dvec,
        nc.semaphore("vfin") as vfin,
        nc.semaphore("odma") as odma,
    ):
        cks = [_sems.enter_context(nc.semaphore(f"ck{c}")) for c in range(len(CHUNKS))]

        # row r of the shard lives at partition r // nt, tile r % nt: each
        # partition's 32 rows are CONSECUTIVE in DRAM, so a chunk load is one
        # contiguous descriptor per partition (128/chunk, not 128/tile) and
        # the stream is bandwidth-bound instead of HWDGE-descriptor-bound
        pred_pm = pred.rearrange("(p k) c -> p k c", p=P)

        def load_chunk(eng, c):
            s, l = CHUNKS[c]
            eng.dma_start(
                out=x_all[:, s:s + l, :], in_=pred_pm[:, s:s + l, :]
            ).then_inc(cks[c], 16)

        @block.sync
        def _(sync):
            load_chunk(sync, 0)
            load_chunk(sync, 1)
            sync.dma_start(out=t_pair, in_=tgt).then_inc(t_sem, 16)
            for c in range(2, len(CHUNKS)):
                load_chunk(sync, c)
            sync.wait_ge(vfin, 1)
            sync.dma_start(out=out, in_=outsb).then_inc(odma, 16)
            sync.wait_ge(odma, 16)

        @block.gpsimd
        def _(gpsimd):
            gpsimd.memset(warm_t, 0.0).then_inc(warm_sem, 1)
            gpsimd.iota(iota_i, pattern=[[1, C]], base=0,
                        channel_multiplier=0).then_inc(g_iota, 1)
            gpsimd.dma_start(out=band_sb, in_=band).then_inc(band_sem, 16)

        @block.scalar
        def _(scalar):
            scalar.wait_ge(warm_sem, 1)
            scalar.activation(out=warm_t[:, 0:1], in_=warm_t[:, 2:3], func=AF.Exp)
            a_idx = 0
            for c, (s, l) in enumerate(CHUNKS):
                scalar.wait_ge(cks[c], 16)
                if c in B_ORD:
                    b = B_ORD[c]
                    slot = (b - 1) % ESLOTS
                    if b > ESLOTS:
                        scalar.wait_ge(bred, b - ESLOTS)
                    scalar.activation(
                        out=esc_b[:, slot, 0:l, :], in_=x_all[:, s:s + l, :],
                        func=AF.Exp,
                    ).then_inc(escb, 1)
                else:
                    for i in range(s, s + l):
                        if a_idx >= 2:
                            # esc_a is double-buffered: <=2 exps in flight
                            scalar.wait_ge(act_sem, a_idx - 1)
                        scalar.activation(
                            out=esc_a[:, a_idx % 2, :], in_=x_all[:, i, :],
                            func=AF.Exp,
                            accum_out=sumexp_sb[:, i:i + 1],
                        ).then_inc(act_sem, 1)
                        a_idx += 1
            scalar.wait_ge(act_sem, a_idx)          # all accumulates landed
            scalar.wait_ge(bred, len(B_CHUNKS))     # all B sums landed
            scalar.activation(out=lse, in_=sumexp_sb, func=AF.Ln).then_inc(
                lnsem, 1)

        @block.vector
        def _(vector):
            ndve = 0

            def dv(ins):
                nonlocal ndve
                ndve += 1
                return ins.then_inc(dvec, 1)

            def dwait():
                vector.wait_ge(dvec, ndve)

            vector.wait_ge(t_sem, 16)
            dv(vector.tensor_copy(out=t_f, in_=t_pair[:, :, 0]))
            vector.wait_ge(g_iota, 1)
            dv(vector.tensor_copy(out=iota_h, in_=iota_i))
            dv(vector.memset(wr, 1.0))
            for d in (1, 2, 3):
                dwait()
                dv(vector.tensor_scalar(
                    out=wtmp, in0=t_f, scalar1=d - 0.5, scalar2=WDEC[d],
                    op0=OP.is_ge, op1=OP.mult))
                dwait()
                dv(vector.tensor_tensor(out=wr, in0=wr, in1=wtmp, op=OP.add))
                dwait()
                dv(vector.tensor_scalar(
                    out=wtmp, in0=t_f, scalar1=(C - 1 - d) + 0.5,
                    scalar2=WDEC[d], op0=OP.is_le, op1=OP.mult))
                dwait()
                dv(vector.tensor_tensor(out=wr, in0=wr, in1=wtmp, op=OP.add))
            dwait()  # preamble fully retired before the loop reads iota_h/t_f

            for c, (s, l) in enumerate(CHUNKS):
                vector.wait_ge(cks[c], 16)
                for i in range(s, s + l):
                    vector.tensor_scalar(
                        out=h_all[:, i, :], in0=iota_h,
                        scalar1=t_f[:, i:i + 1], scalar2=None,
                        op0=OP.is_equal, op1=OP.bypass,
                    ).then_inc(h_dve, 1)
                if c in RED_AT:
                    b = RED_AT[c]
                    bc = B_CHUNKS[b - 1]
                    bs, bl = CHUNKS[bc]
                    vector.wait_ge(escb, b)
                    vector.tensor_reduce(
                        out=sumexp_sb[:, bs:bs + bl],
                        in_=esc_b[:, (b - 1) % ESLOTS, 0:bl, :],
                        axis=mybir.AxisListType.X, op=OP.add,
                    ).then_inc(bred, 1)

            vector.wait_ge(pe_tile, nt)
            vector.wait_ge(band_sem, 16)
            dv(vector.tensor_tensor(
                out=junk, in0=psum[:, :, 0:P], in1=band_sb, op=OP.mult))
            dwait()
            dv(vector.tensor_reduce(
                out=outsb[:, nt:nt + 1],
                in_=junk.rearrange("p b n -> p (b n)"),
                axis=mybir.AxisListType.X, op=OP.add))
            dwait()  # G landed before the out DMA reads outsb
            vector.wait_ge(lnsem, 1)
            vector.tensor_tensor(
                out=outsb[:, 0:nt], in0=wr, in1=lse, op=OP.mult
            ).then_inc(vfin, 1)

        @block.tensor
        def _(pe):
            for i in range(nt):
                c = _chunk_of(i)
                s, l = CHUNKS[c]
                if i == s:
                    pe.wait_ge(cks[c], 16)
                    pe.wait_ge(h_dve, s + l)
                for b in range(NB):
                    bs = BLK[b]
                    mm = pe.matmul(
                        psum[:, b, 0:P], h_all[:, i, bs:bs + P],
                        x_bf[:, i, bs:bs + P, 1],
                        start=(i == 0), stop=(i == nt - 1),
                    )
                mm.then_inc(pe_tile, 1)

    nc.compile()
    return nc


def _shard_inputs(prediction: np.ndarray, target: np.ndarray, rpc: int, ncores: int):
    pred = np.ascontiguousarray(np.asarray(prediction, dtype=np.float32)).reshape(-1, C)
    tgt = np.ascontiguousarray(np.asarray(target)).reshape(-1)
    if tgt.dtype == np.int64:
        tgt_pair = tgt.view(np.int32).reshape(-1, 2)  # little-endian low word first
    else:
        lo = tgt.astype(np.int32, copy=False)
        tgt_pair = np.stack([lo, np.zeros_like(lo)], axis=1)
    nt = rpc // P
    band = _band_masks().reshape(P, NB, P)
    in_maps = []
    for c in range(ncores):
        sl = slice(c * rpc, (c + 1) * rpc)
        in_maps.append({
            "prediction": pred[sl],
            "target_pair": np.ascontiguousarray(tgt_pair[sl].reshape(P, nt, 2)),
            "band": band,
        })
    return in_maps


def _host_combine(results, nt: int) -> np.float32:
    tot = 0.0
    nrows = 0
    for r in results:
        o = np.asarray(r["out"], dtype=np.float64)
        tot += o[:, :nt].sum() - o[:, nt:].sum()
        nrows += P * nt
    return np.float32(tot / nrows)


def kernel(prediction: np.ndarray, target: np.ndarray, _trace: bool = False):
    nc = _build(RPC)
    in_maps = _shard_inputs(prediction, target, RPC, NCORES)
    res = run_bass_kernel_spmd(
        nc, in_maps, core_ids=list(range(NCORES)), trace=_trace
    )
    loss = _host_combine(res.results, RPC // P)
    if _trace:
        return loss, res
    return loss
